# revision 27
# baseline (speedup 1.0000x reference)
"""Causal self-attention (GPT-style block) on 8 Trainium2 NeuronCores.

Sharding: pure data-parallel over batch. B=8 batch elements map 1:1 onto the
8 cores; every core runs the full per-sequence attention, so no collectives.

Host-side prep: x, w_attn, w_proj transposed/cast to bf16 and PACKED into the
exact SBUF layouts the device consumes:
  xT   [C, T]            x^T, 6 contiguous 128-row chunks
  waP  [6, 6, 128, 256]  per head-PAIR stationary weights [q_og | k_og]
  waV  [6, 128, 768]     v columns (all heads) chunked by c
  wpT  [C, C]            w_proj^T

Per-core device program (T=1024, C=768, H=12, hd=64), scheduled so the PE
(the 96us bf16 roofline for this shape) never stalls:
  1. q^T/k^T per pair via matmul (fp32 PSUM, DVE bias-drain to bf16);
     QK(p) staggered one pair behind qkv(p) so drains are hidden.
  2. S^T = k@q^T per head (K=64 row-tiled, heads at partition 0/64); ONE
     exp per (pair,head,tk-chunk) on ACT reading across 2 PSUM banks,
     1/sqrt(hd) folded into the activation scale. Causal masking: skip
     fully-masked chunks + one triangular mask-multiply (on Pool) per
     diagonal block.
  3. v for ALL heads as one GEMM group late (v is only needed by PV);
     v_aug carries a ones column per head for fused softmax denominators.
  4. PV j-major across ALL heads: per (tq-chunk j, 6-head group) one PSUM
     bank accumulates [128, 6x65]; softmax normalize = 1 strided reciprocal
     + 1 broadcast (stride-0) tensor_tensor multiply on DVE. y chunks
     finalize in j order, each DMA-transposed immediately (SP xbar) so the
     projection pipelines right behind PV instead of serializing.
  5. proj streams per tq chunk; DVE bias-drain; output DMA split SP/Pool.

Engine budget (ns, per core): PE ~97k (roofline), ACT ~64k exp + 2.4k DMA,
DVE ~45k drains/normalize, Pool ~30k masks+DMA, SP ~20k DMA+transpose.
"""

import sys
from contextlib import ExitStack

import numpy as np

if "/opt/trn_rl_repo" not in sys.path:
    sys.path.insert(0, "/opt/trn_rl_repo")

import concourse.bacc as bacc
import concourse.bass as bass
import concourse.tile as tile
from concourse import mybir
from concourse.masks import make_upper_triangular

F32 = mybir.dt.float32
BF16 = mybir.dt.bfloat16
FP8 = mybir.dt.float8e4
DR = mybir.MatmulPerfMode.DoubleRow

T = 1024
C = 768
H = 12
HD = C // H  # 64
N_CORES = 8
# w_attn is scaled by 16 on the host before the fp8 hi/lo split so the lo
# residuals stay in e4m3's normal range (w ~ N(0, 1/sqrt(C)) would otherwise
# put them among subnormals); the qkv drains fold in the 1/16.
W8SCALE = 16.0
NT = T // 128  # 8 t-chunks
NCC = C // 128  # 6 c-chunks
NHP = H // 2  # 6 head pairs


def build_attention_core(t=T, repeats=1):
    nc = bacc.Bacc(None, target_bir_lowering=False, debug=False)
    # fp8 hi/lo split operands for the qkv GEMM (DoubleRow: [part, ktile2, .]
    # with contraction row r = 256*chunk + 128*ktile + part)
    xT8_d = nc.declare_dram_parameter("xT8", [2, 3, 128, 2, t], FP8, isOutput=False)
    waP8_d = nc.declare_dram_parameter(
        "waP8", [NHP, 2, 3, 128, 2, 256], FP8, isOutput=False
    )
    waV8_d = nc.declare_dram_parameter("waV8", [2, 3, 128, 2, C], FP8, isOutput=False)
    b_attn = nc.declare_dram_parameter("b_attn", [3 * C], F32, isOutput=False)
    wpT_d = nc.declare_dram_parameter("wpT", [C, C], BF16, isOutput=False)
    b_proj = nc.declare_dram_parameter("b_proj", [C], F32, isOutput=False)
    out = nc.declare_dram_parameter("out", [t, C], F32, isOutput=True)

    with ExitStack() as octx:
        tc = octx.enter_context(tile.TileContext(nc))
        for _rep in range(repeats):
            _emit_once(nc, tc, t, xT8_d, waP8_d, waV8_d, b_attn, wpT_d, b_proj, out)
    nc.compile()
    return nc


def _pieces(total, maxw=512):
    res, s = [], 0
    while s < total:
        w = min(maxw, total - s)
        res.append((s, w))
        s += w
    return res


def _emit_once(nc, tc, t, xT8_d, waP8_d, waV8_d, b_attn, wpT_d, b_proj, out):
    nt = t // 128

    with ExitStack() as ctx:
        singles = ctx.enter_context(tc.tile_pool(name="singles", bufs=1))
        pool = ctx.enter_context(tc.tile_pool(name="main", bufs=1))
        psum = ctx.enter_context(tc.tile_pool(name="psum", bufs=1, space="PSUM"))

        # ---- constants / singles --------------------------------------
        warm = singles.tile([128, 128], BF16, name="warm")
        nc.vector.memset(warm, 0.0)

        tri = singles.tile([128, 128], BF16, name="tri")
        make_upper_triangular(nc, tri, val=1.0, diag=True)

        bias_qk = singles.tile([128, 2 * NCC], F32, name="bias_qk")
        bias_v = singles.tile([128, C], F32, name="bias_v")
        bias_p = singles.tile([128, C], F32, name="bias_p")

        waV8 = singles.tile([128, 2, 3, 2, C], FP8, name="waV8")
        wpT = singles.tile([128, NCC, C], BF16, name="wpT")

        # ---- persistent big tiles -------------------------------------
        # x8's slot is reused by yTall (tag ring, bufs=1): x is dead once
        # the v GEMMs finish, just before the first transpose lands.
        x8 = pool.tile([128, 2, 3, 2, t], FP8, name="x8", tag="big", bufs=1)
        v_aug = [
            pool.tile([128, H, HD + 1], BF16, name=f"vaug{i}") for i in range(nt)
        ]
        y_nat = [pool.tile([128, C], BF16, name=f"ynat{j}") for j in range(nt)]

        for i in range(nt):
            ones = v_aug[i][:, :, HD : HD + 1]
            nc.vector.memset(ones, 1.0)

        # ---- input DMAs (3 queues: SP, ACT, Pool) ---------------------
        # PE's first matmul needs x8-hi (3 chunks) + waP8 pair 0, so those
        # lead the SP queue; lo chunks follow on ACT/Pool.
        waPs = []
        for p in range(NHP):
            waPs.append(
                pool.tile([128, 2, 3, 2, 256], FP8, name=f"waP{p}", tag="waP", bufs=2)
            )
        nc.sync.dma_start(out=x8[:, 0, 0], in_=xT8_d[0, 0])
        nc.sync.dma_start(
            out=waPs[0], in_=waP8_d[0].rearrange("hl c p k o -> p hl c k o")
        )
        nc.sync.dma_start(out=x8[:, 0, 1], in_=xT8_d[0, 1])
        nc.gpsimd.dma_start(out=x8[:, 0, 2], in_=xT8_d[0, 2])
        for c in range(3):
            nc.gpsimd.dma_start(out=x8[:, 1, c], in_=xT8_d[1, c])
        nc.sync.dma_start(
            out=bias_qk, in_=b_attn[0 : 2 * C].rearrange("(c p) -> p c", p=128)
        )
        for p in range(1, NHP):
            nc.sync.dma_start(
                out=waPs[p], in_=waP8_d[p].rearrange("hl c p k o -> p hl c k o")
            )
        for hl in range(2):
            for c in range(3):
                nc.gpsimd.dma_start(out=waV8[:, hl, c], in_=waV8_d[hl, c])
        for cc in range(NCC):
            nc.gpsimd.dma_start(
                out=wpT[:, cc, :], in_=wpT_d[cc * 128 : (cc + 1) * 128, :]
            )
        # bias broadcasts are not needed until the first drains; keep them
        # behind the x8-lo chunks on the Pool queue.
        bav = b_attn[2 * C : 3 * C].rearrange("(o c) -> o c", o=1)
        nc.gpsimd.dma_start(
            out=bias_v,
            in_=bass.AP(tensor=bav.tensor, offset=bav.offset, ap=[[0, 128]] + bav.ap[1:]),
        )
        bpv = b_proj[:].rearrange("(o c) -> o c", o=1)
        nc.gpsimd.dma_start(
            out=bias_p,
            in_=bass.AP(tensor=bpv.tensor, offset=bpv.offset, ap=[[0, 128]] + bpv.ap[1:]),
        )

        # ---- PE warmup (p-state ramp while input DMAs land) -----------
        for _ in range(13):
            pw = psum.tile([128, 512], F32, name="ps_s", tag="s", bufs=2)
            nc.tensor.matmul(pw[:, 0:128], warm, warm, start=True, stop=True)

        qT = {}
        kT = {}
        TERMS = ((0, 0), (1, 0), (0, 1))  # (x_hl, w_hl): hi@hi + lo@hi + hi@lo

        def emit_qk(p):
            # inner order q0,q1,k0,k1: QK(p) i=0 needs q fully + k cols 0:512
            # only, so it can start one DVE-drain after the 3rd group.
            qT[p] = pool.tile([128, t], BF16, name=f"qT{p}", tag="qT", bufs=2)
            kT[p] = pool.tile([128, t], BF16, name=f"kT{p}", tag="kT", bufs=2)
            for qk in range(2):
                dst = qT[p] if qk == 0 else kT[p]
                ogidx = p if qk == 0 else NCC + p
                for (s, w) in _pieces(t):
                    pq = psum.tile([128, 512], F32, name="ps_mm", tag="mm", bufs=4)
                    for ti, (xh, wh) in enumerate(TERMS):
                        for c in range(3):
                            nc.tensor.matmul(
                                pq[:, :w],
                                waPs[p][:, wh, c, :, qk * 128 : qk * 128 + 128],
                                x8[:, xh, c, :, s : s + w],
                                start=(ti == 0 and c == 0),
                                stop=(ti == 2 and c == 2),
                                perf_mode=DR,
                            )
                    nc.vector.tensor_scalar(
                        dst[:, s : s + w],
                        pq[:, :w],
                        1.0 / W8SCALE,
                        bias_qk[:, ogidx : ogidx + 1],
                        op0=mybir.AluOpType.mult,
                        op1=mybir.AluOpType.add,
                    )

        eS = {}

        def emit_qk_s(p, i_range):
            """S^T chunks + exp for pair p over the given tk-chunks.

            e tiles are [128, 2, w] (head-major). i<4 uses one 2-bank psum +
            one exp per head; i>=4 packs both heads into one psum tile
            (banks 0/1) and does a single gap-skipping exp for both.
            """
            for i in i_range:
                w = t - 128 * i
                e = pool.tile([128, 2, w], BF16, name=f"e{p}_{i}")
                eS[(p, i)] = e
                if w > 512:
                    for hh in range(2):
                        ps = psum.tile([128, 1024], F32, name="ps_s", tag="s", bufs=2)
                        for (s, wp) in _pieces(w):
                            tq0 = 128 * i + s
                            nc.tensor.matmul(
                                ps[:, s : s + wp],
                                kT[p][hh * 64 : hh * 64 + 64, 128 * i : 128 * (i + 1)],
                                qT[p][hh * 64 : hh * 64 + 64, tq0 : tq0 + wp],
                                start=True,
                                stop=True,
                            )
                        nc.scalar.activation(
                            e[:, hh, :],
                            ps[:, 0:w],
                            mybir.ActivationFunctionType.Exp,
                            bias=0.0,
                            scale=1.0 / float(np.sqrt(HD)),
                        )
                else:
                    ps = psum.tile([128, 1024], F32, name="ps_s", tag="s", bufs=2)
                    for hh in range(2):
                        nc.tensor.matmul(
                            ps[:, hh * 512 : hh * 512 + w],
                            kT[p][hh * 64 : hh * 64 + 64, 128 * i : 128 * (i + 1)],
                            qT[p][hh * 64 : hh * 64 + 64, 128 * i : t],
                            start=True,
                            stop=True,
                        )
                    nc.scalar.activation(
                        e,
                        bass.AP(
                            tensor=ps.tensor, offset=ps.offset, ap=[ps.ap[0], [512, 2], [1, w]]
                        ),
                        mybir.ActivationFunctionType.Exp,
                        bias=0.0,
                        scale=1.0 / float(np.sqrt(HD)),
                    )
                # causal mask on the diagonal 128x128 block (keep tk<=tq)
                for hh in range(2):
                    nc.gpsimd.tensor_mul(e[:, hh, 0:128], e[:, hh, 0:128], tri)

        def emit_v(it):
            for (s, w) in _pieces(C):
                pv = psum.tile([128, 512], F32, name="ps_mm", tag="mm", bufs=4)
                for ti, (xh, wh) in enumerate(TERMS):
                    for c in range(3):
                        nc.tensor.matmul(
                            pv[:, :w],
                            x8[:, xh, c, :, it * 128 : (it + 1) * 128],
                            waV8[:, wh, c, :, s : s + w],
                            start=(ti == 0 and c == 0),
                            stop=(ti == 2 and c == 2),
                            perf_mode=DR,
                        )
                nh = w // HD
                h0 = s // HD
                nc.vector.scalar_tensor_tensor(
                    out=v_aug[it][:, h0 : h0 + nh, 0:HD],
                    in0=pv[:, :w].rearrange("p (h d) -> p h d", d=HD),
                    scalar=1.0 / W8SCALE,
                    in1=bias_v[:, s : s + w].rearrange("p (h d) -> p h d", d=HD),
                    op0=mybir.AluOpType.mult,
                    op1=mybir.AluOpType.add,
                )

        # Schedule: exp starts ASAP (QK(0) i=0,1 right after qk(0)); qk/v
        # GEMMs are scattered between QK phases as PE filler so the PE tracks
        # ACT's exp pace (~10us/pair) without stalling on the ps_s ring.
        emit_qk(0)
        emit_qk_s(0, range(0, 2))
        emit_qk(1)
        emit_qk_s(0, range(2, nt))
        emit_qk(2)
        emit_qk_s(1, range(nt))
        emit_v(0)
        emit_v(1)
        emit_qk(3)
        emit_qk_s(2, range(nt))
        emit_v(2)
        emit_v(3)
        emit_qk(4)
        emit_qk_s(3, range(nt))
        emit_v(4)
        emit_v(5)
        emit_qk(5)
        emit_qk_s(4, range(nt))
        emit_v(6)
        emit_v(7)

        # ---- PV (j-major) + projection, software-pipelined ------------
        yTall = pool.tile([128, NCC, t], BF16, name="yTall", tag="big", bufs=1)

        def emit_pv(j, half):
            py = psum.tile([128, 512], F32, name="ps_y", tag="mm", bufs=4)
            for hl in range(6):
                head = 6 * half + hl
                p, hh = head // 2, head % 2
                for i in range(j + 1):
                    nc.tensor.matmul(
                        py[:, hl * 65 : hl * 65 + 65],
                        eS[(p, i)][:, hh, 128 * (j - i) : 128 * (j - i) + 128],
                        v_aug[i][:, head, :],
                        start=(i == 0),
                        stop=(i == j),
                    )
            rcp = singles.tile([128, 6], F32, name="rcp", tag="rcp", bufs=4)
            nc.vector.reciprocal(
                rcp,
                bass.AP(tensor=py.tensor, offset=py.offset + HD, ap=[py.ap[0], [65, 6]]),
            )
            nc.vector.tensor_tensor(
                out=bass.AP(
                    tensor=y_nat[j].tensor,
                    offset=y_nat[j].offset + half * 384,
                    ap=[y_nat[j].ap[0], [64, 6], [1, 64]],
                ),
                in0=bass.AP(
                    tensor=py.tensor, offset=py.offset, ap=[py.ap[0], [65, 6], [1, 64]]
                ),
                in1=bass.AP(
                    tensor=rcp.tensor, offset=rcp.offset, ap=[rcp.ap[0], [1, 6], [0, 64]]
                ),
                op=mybir.AluOpType.mult,
            )

        def emit_proj(it):
            out_sb = singles.tile([128, C], F32, name="out_sb", tag="osb", bufs=2)
            for pi, (s, w) in enumerate(_pieces(C)):
                po = psum.tile([128, 512], F32, name="ps_o", tag="mm", bufs=4)
                for cc in range(NCC):
                    nc.tensor.matmul(
                        po[:, :w],
                        yTall[:, cc, it * 128 : (it + 1) * 128],
                        wpT[:, cc, s : s + w],
                        start=(cc == 0),
                        stop=(cc == NCC - 1),
                    )
                nc.vector.tensor_add(
                    out_sb[:, s : s + w], po[:, :w], bias_p[:, s : s + w]
                )
                # drain each piece to HBM as soon as it lands (SP/Pool split)
                eng = nc.sync if pi == 0 else nc.gpsimd
                eng.dma_start(
                    out=out[it * 128 : (it + 1) * 128, s : s + w],
                    in_=out_sb[:, s : s + w],
                )

        # QK(5)'s ps_s ring drains at ACT's exp pace, so interleave the PV
        # groups of heads 0-5 (pairs 0-2, whose exps are long done) as PE
        # filler; then pipeline pair-5-dependent PV + projection, with proj
        # staggered 3 behind PVh1 to cover the normalize+transpose latency.
        for jj in range(4):
            emit_qk_s(5, range(2 * jj, 2 * jj + 2))
            emit_pv(2 * jj, 0)
            emit_pv(2 * jj + 1, 0)
        for j in range(nt):
            emit_pv(j, 1)
            nc.sync.dma_start_transpose(
                yTall[:, :, j * 128 : (j + 1) * 128], y_nat[j]
            )
            if j >= 3:
                emit_proj(j - 3)
        for it in range(nt - 3, nt):
            emit_proj(it)


_NC_CACHE = {}


def get_nc(t=T):
    if t not in _NC_CACHE:
        _NC_CACHE[t] = build_attention_core(t)
    return _NC_CACHE[t]


def _to_bf16(a):
    import ml_dtypes

    return np.ascontiguousarray(np.asarray(a, dtype=np.float32)).astype(
        ml_dtypes.bfloat16
    )


def _split_fp8(a):
    """hi/lo error-compensated fp8e4 split: a ~= hi + lo."""
    import ml_dtypes

    f8 = ml_dtypes.float8_e4m3
    hi = a.astype(f8)
    lo = (a - hi.astype(np.float32)).astype(f8)
    return hi, lo


def _pack_dr(a, ncols):
    """[C, ncols] fp8 -> [3, 128, 2, ncols] DoubleRow layout
    (contraction row r = 256*chunk + 128*ktile + part)."""
    return np.ascontiguousarray(a.reshape(3, 2, 128, ncols).transpose(0, 2, 1, 3))


def host_prep(inputs):
    """Transpose + cast + pack weights/x on the host for the device program."""
    x = np.asarray(inputs["x"], dtype=np.float32)
    b_attn = np.ascontiguousarray(inputs["b_attn"], dtype=np.float32)
    b_proj = np.ascontiguousarray(inputs["b_proj"], dtype=np.float32)
    waT = np.asarray(inputs["w_attn"], dtype=np.float32).T * W8SCALE  # [C, 3C]
    waP8 = np.stack(
        [
            np.stack(
                [
                    _pack_dr(h, 256)
                    for h in _split_fp8(
                        np.concatenate(
                            [
                                waT[:, p * 128 : (p + 1) * 128],
                                waT[:, (NCC + p) * 128 : (NCC + p + 1) * 128],
                            ],
                            axis=1,
                        )
                    )
                ]
            )
            for p in range(NHP)
        ]
    )  # [6, 2, 3, 128, 2, 256]
    waV8 = np.stack([_pack_dr(h, C) for h in _split_fp8(waT[:, 2 * C : 3 * C])])
    wpT = _to_bf16(np.asarray(inputs["w_proj"], dtype=np.float32).T)  # [C, C]
    return [
        {
            "xT8": np.stack(
                [_pack_dr(h, x.shape[1]) for h in _split_fp8(x[b].T)]
            ),  # [2, 3, 128, 2, T]
            "waP8": waP8,
            "waV8": waV8,
            "b_attn": b_attn,
            "wpT": wpT,
            "b_proj": b_proj,
        }
        for b in range(x.shape[0])
    ]


def kernel(**inputs):
    from concourse.bass_utils import run_bass_kernel_spmd

    x = inputs["x"]
    B, t, _ = x.shape
    assert B == N_CORES
    in_maps = host_prep(inputs)
    nc = get_nc(t)
    res = run_bass_kernel_spmd(nc, in_maps, core_ids=list(range(N_CORES)))
    return np.stack([res.results[b]["out"] for b in range(B)]).astype(np.float32)


# revision 28
# speedup vs baseline: 1.0064x; 1.0064x over previous
"""Causal self-attention (GPT-style block) on 8 Trainium2 NeuronCores.

Sharding: pure data-parallel over batch. B=8 batch elements map 1:1 onto the
8 cores; every core runs the full per-sequence attention, so no collectives.

Host-side prep: x, w_attn, w_proj transposed/cast to bf16 and PACKED into the
exact SBUF layouts the device consumes:
  xT   [C, T]            x^T, 6 contiguous 128-row chunks
  waP  [6, 6, 128, 256]  per head-PAIR stationary weights [q_og | k_og]
  waV  [6, 128, 768]     v columns (all heads) chunked by c
  wpT  [C, C]            w_proj^T

Per-core device program (T=1024, C=768, H=12, hd=64), scheduled so the PE
(the 96us bf16 roofline for this shape) never stalls:
  1. q^T/k^T per pair via matmul (fp32 PSUM, DVE bias-drain to bf16);
     QK(p) staggered one pair behind qkv(p) so drains are hidden.
  2. S^T = k@q^T per head (K=64 row-tiled, heads at partition 0/64); ONE
     exp per (pair,head,tk-chunk) on ACT reading across 2 PSUM banks,
     1/sqrt(hd) folded into the activation scale. Causal masking: skip
     fully-masked chunks + one triangular mask-multiply (on Pool) per
     diagonal block.
  3. v for ALL heads as one GEMM group late (v is only needed by PV);
     v_aug carries a ones column per head for fused softmax denominators.
  4. PV j-major across ALL heads: per (tq-chunk j, 6-head group) one PSUM
     bank accumulates [128, 6x65]; softmax normalize = 1 strided reciprocal
     + 1 broadcast (stride-0) tensor_tensor multiply on DVE. y chunks
     finalize in j order, each DMA-transposed immediately (SP xbar) so the
     projection pipelines right behind PV instead of serializing.
  5. proj streams per tq chunk; DVE bias-drain; output DMA split SP/Pool.

Engine budget (ns, per core): PE ~97k (roofline), ACT ~64k exp + 2.4k DMA,
DVE ~45k drains/normalize, Pool ~30k masks+DMA, SP ~20k DMA+transpose.
"""

import sys
from contextlib import ExitStack

import numpy as np

if "/opt/trn_rl_repo" not in sys.path:
    sys.path.insert(0, "/opt/trn_rl_repo")

import concourse.bacc as bacc
import concourse.bass as bass
import concourse.tile as tile
from concourse import mybir
from concourse.masks import make_upper_triangular

F32 = mybir.dt.float32
BF16 = mybir.dt.bfloat16
FP8 = mybir.dt.float8e4
DR = mybir.MatmulPerfMode.DoubleRow

T = 1024
C = 768
H = 12
HD = C // H  # 64
N_CORES = 8
# w_attn is scaled by 16 on the host before the fp8 hi/lo split so the lo
# residuals stay in e4m3's normal range (w ~ N(0, 1/sqrt(C)) would otherwise
# put them among subnormals); the qkv drains fold in the 1/16.
W8SCALE = 16.0
NT = T // 128  # 8 t-chunks
NCC = C // 128  # 6 c-chunks
NHP = H // 2  # 6 head pairs


def build_attention_core(t=T, repeats=1):
    nc = bacc.Bacc(None, target_bir_lowering=False, debug=False)
    # fp8 hi/lo split operands for the qkv GEMM (DoubleRow: [part, ktile2, .]
    # with contraction row r = 256*chunk + 128*ktile + part)
    xT8_d = nc.declare_dram_parameter("xT8", [2, 3, 128, 2, t], FP8, isOutput=False)
    waP8_d = nc.declare_dram_parameter(
        "waP8", [NHP, 2, 3, 128, 512], FP8, isOutput=False
    )
    waV8_d = nc.declare_dram_parameter("waV8", [2, 3, 128, 2, C], FP8, isOutput=False)
    b_attn = nc.declare_dram_parameter("b_attn", [3 * C], F32, isOutput=False)
    wpT_d = nc.declare_dram_parameter("wpT", [C, C], BF16, isOutput=False)
    b_proj = nc.declare_dram_parameter("b_proj", [C], F32, isOutput=False)
    out = nc.declare_dram_parameter("out", [t, C], F32, isOutput=True)

    with ExitStack() as octx:
        tc = octx.enter_context(tile.TileContext(nc))
        for _rep in range(repeats):
            _emit_once(nc, tc, t, xT8_d, waP8_d, waV8_d, b_attn, wpT_d, b_proj, out)
    nc.compile()
    return nc


def _pieces(total, maxw=512):
    res, s = [], 0
    while s < total:
        w = min(maxw, total - s)
        res.append((s, w))
        s += w
    return res


def _emit_once(nc, tc, t, xT8_d, waP8_d, waV8_d, b_attn, wpT_d, b_proj, out):
    nt = t // 128

    with ExitStack() as ctx:
        singles = ctx.enter_context(tc.tile_pool(name="singles", bufs=1))
        pool = ctx.enter_context(tc.tile_pool(name="main", bufs=1))
        psum = ctx.enter_context(tc.tile_pool(name="psum", bufs=1, space="PSUM"))

        # ---- constants / singles --------------------------------------
        warm = singles.tile([128, 128], BF16, name="warm")
        nc.vector.memset(warm, 0.0)

        tri = singles.tile([128, 128], BF16, name="tri")
        make_upper_triangular(nc, tri, val=1.0, diag=True)

        bias_qk = singles.tile([128, 2 * NCC], F32, name="bias_qk")
        bias_v = singles.tile([128, C], F32, name="bias_v")
        bias_p = singles.tile([128, C], F32, name="bias_p")

        waV8 = singles.tile([128, 2, 3, 2, C], FP8, name="waV8")
        wpT = singles.tile([128, NCC, C], BF16, name="wpT")

        # ---- persistent big tiles -------------------------------------
        # x8's slot is reused by yTall (tag ring, bufs=1): x is dead once
        # the v GEMMs finish, just before the first transpose lands.
        x8 = pool.tile([128, 2, 3, 2, t], FP8, name="x8", tag="big", bufs=1)
        v_aug = [
            pool.tile([128, H, HD + 1], BF16, name=f"vaug{i}") for i in range(nt)
        ]
        y_nat = [pool.tile([128, C], BF16, name=f"ynat{j}") for j in range(nt)]

        for i in range(nt):
            ones = v_aug[i][:, :, HD : HD + 1]
            nc.vector.memset(ones, 1.0)

        # ---- input DMAs (3 queues: SP, ACT, Pool) ---------------------
        # PE's first matmul needs x8-hi (3 chunks) + waP8 pair 0, so those
        # lead the SP queue; lo chunks follow on ACT/Pool.
        waPs = []
        for p in range(NHP):
            waPs.append(
                pool.tile([128, 2, 3, 512], FP8, name=f"waP{p}", tag="waP", bufs=2)
            )
        nc.sync.dma_start(out=x8[:, 0, 0], in_=xT8_d[0, 0])
        nc.sync.dma_start(
            out=waPs[0], in_=waP8_d[0].rearrange("hl c p o -> p hl c o")
        )
        nc.sync.dma_start(out=x8[:, 0, 1], in_=xT8_d[0, 1])
        nc.scalar.dma_start(out=x8[:, 0, 2], in_=xT8_d[0, 2])
        for c in range(3):
            nc.gpsimd.dma_start(out=x8[:, 1, c], in_=xT8_d[1, c])
        nc.sync.dma_start(
            out=bias_qk, in_=b_attn[0 : 2 * C].rearrange("(c p) -> p c", p=128)
        )
        for p in range(1, NHP):
            nc.sync.dma_start(
                out=waPs[p], in_=waP8_d[p].rearrange("hl c p o -> p hl c o")
            )
        for hl in range(2):
            for c in range(3):
                nc.gpsimd.dma_start(out=waV8[:, hl, c], in_=waV8_d[hl, c])
        for cc in range(NCC):
            nc.gpsimd.dma_start(
                out=wpT[:, cc, :], in_=wpT_d[cc * 128 : (cc + 1) * 128, :]
            )
        # bias broadcasts are not needed until the first drains; keep them
        # behind the x8-lo chunks on the Pool queue.
        bav = b_attn[2 * C : 3 * C].rearrange("(o c) -> o c", o=1)
        nc.gpsimd.dma_start(
            out=bias_v,
            in_=bass.AP(tensor=bav.tensor, offset=bav.offset, ap=[[0, 128]] + bav.ap[1:]),
        )
        bpv = b_proj[:].rearrange("(o c) -> o c", o=1)
        nc.gpsimd.dma_start(
            out=bias_p,
            in_=bass.AP(tensor=bpv.tensor, offset=bpv.offset, ap=[[0, 128]] + bpv.ap[1:]),
        )

        # ---- PE warmup (p-state ramp while input DMAs land) -----------
        for _ in range(24):
            pw = psum.tile([128, 512], F32, name="ps_s", tag="s", bufs=2)
            nc.tensor.matmul(pw[:, 0:128], warm, warm, start=True, stop=True)

        qT = {}
        kT = {}
        TERMS = ((0, 0), (1, 0), (0, 1))  # (x_hl, w_hl): hi@hi + lo@hi + hi@lo

        def emit_qk(p):
            # inner order q0,q1,k0,k1: QK(p) i=0 needs q fully + k cols 0:512
            # only, so it can start one DVE-drain after the 3rd group.
            qT[p] = pool.tile([128, t], BF16, name=f"qT{p}", tag="qT", bufs=2)
            kT[p] = pool.tile([128, t], BF16, name=f"kT{p}", tag="kT", bufs=2)
            for qk in range(2):
                dst = qT[p] if qk == 0 else kT[p]
                ogidx = p if qk == 0 else NCC + p
                for (s, w) in _pieces(t):
                    pq = psum.tile([128, 512], F32, name="ps_mm", tag="mm", bufs=4)
                    for ti, (xh, wh) in enumerate(TERMS):
                        for c in range(3):
                            wsl = waPs[p][:, wh, c, :]
                            nc.tensor.matmul(
                                pq[:, :w],
                                bass.AP(
                                    tensor=wsl.tensor,
                                    offset=wsl.offset + qk * 128,
                                    ap=[wsl.ap[0], [256, 2], [1, 128]],
                                ),
                                x8[:, xh, c, :, s : s + w],
                                start=(ti == 0 and c == 0),
                                stop=(ti == 2 and c == 2),
                                perf_mode=DR,
                            )
                    nc.vector.tensor_scalar(
                        dst[:, s : s + w],
                        pq[:, :w],
                        1.0 / W8SCALE,
                        bias_qk[:, ogidx : ogidx + 1],
                        op0=mybir.AluOpType.mult,
                        op1=mybir.AluOpType.add,
                    )

        eS = {}

        def emit_qk_s(p, i_range):
            """S^T chunks + exp for pair p over the given tk-chunks.

            e tiles are [128, 2, w] (head-major). i<4 uses one 2-bank psum +
            one exp per head; i>=4 packs both heads into one psum tile
            (banks 0/1) and does a single gap-skipping exp for both.
            """
            for i in i_range:
                w = t - 128 * i
                e = pool.tile([128, 2, w], BF16, name=f"e{p}_{i}")
                eS[(p, i)] = e
                if w > 512:
                    for hh in range(2):
                        ps = psum.tile([128, 1024], F32, name="ps_s", tag="s", bufs=2)
                        for (s, wp) in _pieces(w):
                            tq0 = 128 * i + s
                            nc.tensor.matmul(
                                ps[:, s : s + wp],
                                kT[p][hh * 64 : hh * 64 + 64, 128 * i : 128 * (i + 1)],
                                qT[p][hh * 64 : hh * 64 + 64, tq0 : tq0 + wp],
                                start=True,
                                stop=True,
                            )
                        nc.scalar.activation(
                            e[:, hh, :],
                            ps[:, 0:w],
                            mybir.ActivationFunctionType.Exp,
                            bias=0.0,
                            scale=1.0 / float(np.sqrt(HD)),
                        )
                else:
                    ps = psum.tile([128, 1024], F32, name="ps_s", tag="s", bufs=2)
                    for hh in range(2):
                        nc.tensor.matmul(
                            ps[:, hh * 512 : hh * 512 + w],
                            kT[p][hh * 64 : hh * 64 + 64, 128 * i : 128 * (i + 1)],
                            qT[p][hh * 64 : hh * 64 + 64, 128 * i : t],
                            start=True,
                            stop=True,
                        )
                    nc.scalar.activation(
                        e,
                        bass.AP(
                            tensor=ps.tensor, offset=ps.offset, ap=[ps.ap[0], [512, 2], [1, w]]
                        ),
                        mybir.ActivationFunctionType.Exp,
                        bias=0.0,
                        scale=1.0 / float(np.sqrt(HD)),
                    )
                # causal mask on the diagonal 128x128 block (keep tk<=tq)
                for hh in range(2):
                    nc.gpsimd.tensor_mul(e[:, hh, 0:128], e[:, hh, 0:128], tri)

        def emit_v(it):
            for (s, w) in _pieces(C):
                pv = psum.tile([128, 512], F32, name="ps_mm", tag="mm", bufs=4)
                for ti, (xh, wh) in enumerate(TERMS):
                    for c in range(3):
                        nc.tensor.matmul(
                            pv[:, :w],
                            x8[:, xh, c, :, it * 128 : (it + 1) * 128],
                            waV8[:, wh, c, :, s : s + w],
                            start=(ti == 0 and c == 0),
                            stop=(ti == 2 and c == 2),
                            perf_mode=DR,
                        )
                nh = w // HD
                h0 = s // HD
                nc.vector.scalar_tensor_tensor(
                    out=v_aug[it][:, h0 : h0 + nh, 0:HD],
                    in0=pv[:, :w].rearrange("p (h d) -> p h d", d=HD),
                    scalar=1.0 / W8SCALE,
                    in1=bias_v[:, s : s + w].rearrange("p (h d) -> p h d", d=HD),
                    op0=mybir.AluOpType.mult,
                    op1=mybir.AluOpType.add,
                )

        # Schedule: exp starts ASAP (QK(0) i=0,1 right after qk(0)); qk/v
        # GEMMs are scattered between QK phases as PE filler so the PE tracks
        # ACT's exp pace (~10us/pair) without stalling on the ps_s ring.
        emit_qk(0)
        emit_qk_s(0, range(0, 2))
        emit_qk(1)
        emit_qk_s(0, range(2, nt))
        emit_qk(2)
        emit_qk_s(1, range(nt))
        emit_v(0)
        emit_v(1)
        emit_qk(3)
        emit_qk_s(2, range(nt))
        emit_v(2)
        emit_v(3)
        emit_qk(4)
        emit_qk_s(3, range(nt))
        emit_v(4)
        emit_v(5)
        emit_qk(5)
        emit_qk_s(4, range(nt))
        emit_v(6)
        emit_v(7)

        # ---- PV (j-major) + projection, software-pipelined ------------
        yTall = pool.tile([128, NCC, t], BF16, name="yTall", tag="big", bufs=1)

        def emit_pv(j, half):
            py = psum.tile([128, 512], F32, name="ps_y", tag="mm", bufs=4)
            for hl in range(6):
                head = 6 * half + hl
                p, hh = head // 2, head % 2
                for i in range(j + 1):
                    nc.tensor.matmul(
                        py[:, hl * 65 : hl * 65 + 65],
                        eS[(p, i)][:, hh, 128 * (j - i) : 128 * (j - i) + 128],
                        v_aug[i][:, head, :],
                        start=(i == 0),
                        stop=(i == j),
                    )
            rcp = singles.tile([128, 6], F32, name="rcp", tag="rcp", bufs=4)
            nc.vector.reciprocal(
                rcp,
                bass.AP(tensor=py.tensor, offset=py.offset + HD, ap=[py.ap[0], [65, 6]]),
            )
            nc.vector.tensor_tensor(
                out=bass.AP(
                    tensor=y_nat[j].tensor,
                    offset=y_nat[j].offset + half * 384,
                    ap=[y_nat[j].ap[0], [64, 6], [1, 64]],
                ),
                in0=bass.AP(
                    tensor=py.tensor, offset=py.offset, ap=[py.ap[0], [65, 6], [1, 64]]
                ),
                in1=bass.AP(
                    tensor=rcp.tensor, offset=rcp.offset, ap=[rcp.ap[0], [1, 6], [0, 64]]
                ),
                op=mybir.AluOpType.mult,
            )

        def emit_proj(it):
            out_sb = singles.tile([128, C], F32, name="out_sb", tag="osb", bufs=2)
            for pi, (s, w) in enumerate(_pieces(C)):
                po = psum.tile([128, 512], F32, name="ps_o", tag="mm", bufs=4)
                for cc in range(NCC):
                    nc.tensor.matmul(
                        po[:, :w],
                        yTall[:, cc, it * 128 : (it + 1) * 128],
                        wpT[:, cc, s : s + w],
                        start=(cc == 0),
                        stop=(cc == NCC - 1),
                    )
                nc.vector.tensor_add(
                    out_sb[:, s : s + w], po[:, :w], bias_p[:, s : s + w]
                )
                # drain each piece to HBM as soon as it lands (SP/Pool split)
                eng = nc.sync if pi == 0 else nc.gpsimd
                eng.dma_start(
                    out=out[it * 128 : (it + 1) * 128, s : s + w],
                    in_=out_sb[:, s : s + w],
                )

        # QK(5)'s ps_s ring drains at ACT's exp pace, so interleave the PV
        # groups of heads 0-5 (pairs 0-2, whose exps are long done) as PE
        # filler; then pipeline pair-5-dependent PV + projection, with proj
        # staggered 3 behind PVh1 to cover the normalize+transpose latency.
        for jj in range(4):
            emit_qk_s(5, range(2 * jj, 2 * jj + 2))
            emit_pv(2 * jj, 0)
            emit_pv(2 * jj + 1, 0)
        for j in range(nt):
            emit_pv(j, 1)
            nc.sync.dma_start_transpose(
                yTall[:, :, j * 128 : (j + 1) * 128], y_nat[j]
            )
            if j >= 3:
                emit_proj(j - 3)
        for it in range(nt - 3, nt):
            emit_proj(it)


_NC_CACHE = {}


def get_nc(t=T):
    if t not in _NC_CACHE:
        _NC_CACHE[t] = build_attention_core(t)
    return _NC_CACHE[t]


def _to_bf16(a):
    import ml_dtypes

    return np.ascontiguousarray(np.asarray(a, dtype=np.float32)).astype(
        ml_dtypes.bfloat16
    )


def _split_fp8(a):
    """hi/lo error-compensated fp8e4 split: a ~= hi + lo."""
    import ml_dtypes

    f8 = ml_dtypes.float8_e4m3
    hi = a.astype(f8)
    lo = (a - hi.astype(np.float32)).astype(f8)
    return hi, lo


def _pack_dr(a, ncols):
    """[C, ncols] fp8 -> [3, 128, 2, ncols] DoubleRow layout
    (contraction row r = 256*chunk + 128*ktile + part)."""
    return np.ascontiguousarray(a.reshape(3, 2, 128, ncols).transpose(0, 2, 1, 3))


def host_prep(inputs):
    """Transpose + cast + pack weights/x on the host for the device program."""
    x = np.asarray(inputs["x"], dtype=np.float32)
    b_attn = np.ascontiguousarray(inputs["b_attn"], dtype=np.float32)
    b_proj = np.ascontiguousarray(inputs["b_proj"], dtype=np.float32)
    waT = np.asarray(inputs["w_attn"], dtype=np.float32).T * W8SCALE  # [C, 3C]
    waP8 = np.stack(
        [
            np.stack(
                [
                    _pack_dr(h, 256).reshape(3, 128, 512)
                    for h in _split_fp8(
                        np.concatenate(
                            [
                                waT[:, p * 128 : (p + 1) * 128],
                                waT[:, (NCC + p) * 128 : (NCC + p + 1) * 128],
                            ],
                            axis=1,
                        )
                    )
                ]
            )
            for p in range(NHP)
        ]
    )  # [6, 2, 3, 128, 2, 256]
    waV8 = np.stack([_pack_dr(h, C) for h in _split_fp8(waT[:, 2 * C : 3 * C])])
    wpT = _to_bf16(np.asarray(inputs["w_proj"], dtype=np.float32).T)  # [C, C]
    return [
        {
            "xT8": np.stack(
                [_pack_dr(h, x.shape[1]) for h in _split_fp8(x[b].T)]
            ),  # [2, 3, 128, 2, T]
            "waP8": waP8,
            "waV8": waV8,
            "b_attn": b_attn,
            "wpT": wpT,
            "b_proj": b_proj,
        }
        for b in range(x.shape[0])
    ]


def kernel(**inputs):
    from concourse.bass_utils import run_bass_kernel_spmd

    x = inputs["x"]
    B, t, _ = x.shape
    assert B == N_CORES
    in_maps = host_prep(inputs)
    nc = get_nc(t)
    res = run_bass_kernel_spmd(nc, in_maps, core_ids=list(range(N_CORES)))
    return np.stack([res.results[b]["out"] for b in range(B)]).astype(np.float32)


# revision 29
# speedup vs baseline: 1.0509x; 1.0442x over previous
"""Causal self-attention (GPT-style block) on 8 Trainium2 NeuronCores.

Sharding: pure data-parallel over batch. B=8 batch elements map 1:1 onto the
8 cores; every core runs the full per-sequence attention, so no collectives.

Host-side prep: x, w_attn, w_proj transposed/cast to bf16 and PACKED into the
exact SBUF layouts the device consumes:
  xT   [C, T]            x^T, 6 contiguous 128-row chunks
  waP  [6, 6, 128, 256]  per head-PAIR stationary weights [q_og | k_og]
  waV  [6, 128, 768]     v columns (all heads) chunked by c
  wpT  [C, C]            w_proj^T

Per-core device program (T=1024, C=768, H=12, hd=64), scheduled so the PE
(the 96us bf16 roofline for this shape) never stalls:
  1. q^T/k^T per pair via matmul (fp32 PSUM, DVE bias-drain to bf16);
     QK(p) staggered one pair behind qkv(p) so drains are hidden.
  2. S^T = k@q^T per head (K=64 row-tiled, heads at partition 0/64); ONE
     exp per (pair,head,tk-chunk) on ACT reading across 2 PSUM banks,
     1/sqrt(hd) folded into the activation scale. Causal masking: skip
     fully-masked chunks + one triangular mask-multiply (on Pool) per
     diagonal block.
  3. v for ALL heads as one GEMM group late (v is only needed by PV);
     v_aug carries a ones column per head for fused softmax denominators.
  4. PV j-major across ALL heads: per (tq-chunk j, 6-head group) one PSUM
     bank accumulates [128, 6x65]; softmax normalize = 1 strided reciprocal
     + 1 broadcast (stride-0) tensor_tensor multiply on DVE. y chunks
     finalize in j order, each DMA-transposed immediately (SP xbar) so the
     projection pipelines right behind PV instead of serializing.
  5. proj streams per tq chunk; DVE bias-drain; output DMA split SP/Pool.

Engine budget (ns, per core): PE ~97k (roofline), ACT ~64k exp + 2.4k DMA,
DVE ~45k drains/normalize, Pool ~30k masks+DMA, SP ~20k DMA+transpose.
"""

import sys
from contextlib import ExitStack

import numpy as np

if "/opt/trn_rl_repo" not in sys.path:
    sys.path.insert(0, "/opt/trn_rl_repo")

import concourse.bacc as bacc
import concourse.bass as bass
import concourse.tile as tile
from concourse import mybir
from concourse.masks import make_upper_triangular

F32 = mybir.dt.float32
BF16 = mybir.dt.bfloat16
FP8 = mybir.dt.float8e4
DR = mybir.MatmulPerfMode.DoubleRow

T = 1024
C = 768
H = 12
HD = C // H  # 64
N_CORES = 8
# w_attn is scaled by 16 on the host before the fp8 hi/lo split so the lo
# residuals stay in e4m3's normal range (w ~ N(0, 1/sqrt(C)) would otherwise
# put them among subnormals); the qkv drains fold in the 1/16.
W8SCALE = 16.0
NT = T // 128  # 8 t-chunks
NCC = C // 128  # 6 c-chunks
NHP = H // 2  # 6 head pairs


def build_attention_core(t=T, repeats=1, nobias=True):
    nc = bacc.Bacc(None, target_bir_lowering=False, debug=False)
    # fp8 hi/lo split operands for the qkv GEMM (DoubleRow: [part, ktile2, .]
    # with contraction row r = 256*chunk + 128*ktile + part)
    xT8_d = nc.declare_dram_parameter("xT8", [2, 3, 128, 2, t], FP8, isOutput=False)
    waP8_d = nc.declare_dram_parameter(
        "waP8", [NHP, 2, 3, 128, 512], FP8, isOutput=False
    )
    waV8_d = nc.declare_dram_parameter("waV8", [2, 3, 128, 2, C], FP8, isOutput=False)
    b_attn = nc.declare_dram_parameter("b_attn", [3 * C], F32, isOutput=False)
    wpT8_d = nc.declare_dram_parameter("wpT8", [2, 3, 128, 2, C], FP8, isOutput=False)
    b_proj = nc.declare_dram_parameter("b_proj", [C], F32, isOutput=False)
    out = nc.declare_dram_parameter("out", [t, C], F32, isOutput=True)

    with ExitStack() as octx:
        tc = octx.enter_context(tile.TileContext(nc))
        for _rep in range(repeats):
            _emit_once(nc, tc, t, xT8_d, waP8_d, waV8_d, b_attn, wpT8_d, b_proj, out, nobias)
    nc.compile()
    return nc


def _pieces(total, maxw=512):
    res, s = [], 0
    while s < total:
        w = min(maxw, total - s)
        res.append((s, w))
        s += w
    return res


def _emit_once(nc, tc, t, xT8_d, waP8_d, waV8_d, b_attn, wpT8_d, b_proj, out, nobias=True):
    nt = t // 128

    with ExitStack() as ctx:
        singles = ctx.enter_context(tc.tile_pool(name="singles", bufs=1))
        pool = ctx.enter_context(tc.tile_pool(name="main", bufs=1))
        psum = ctx.enter_context(tc.tile_pool(name="psum", bufs=1, space="PSUM"))

        # ---- constants / singles --------------------------------------
        warm = singles.tile([128, 128], BF16, name="warm")
        nc.vector.memset(warm, 0.0)

        tri = singles.tile([128, 128], BF16, name="tri")
        make_upper_triangular(nc, tri, val=1.0, diag=True)

        bias_qk = singles.tile([128, 2 * NCC], F32, name="bias_qk")
        bias_v = singles.tile([128, C], F32, name="bias_v")
        bias_p = singles.tile([128, C], F32, name="bias_p")

        waV8 = singles.tile([128, 2, 3, 2, C], FP8, name="waV8")
        wpT8 = singles.tile([128, 2, 3, 2, C], FP8, name="wpT8")

        # ---- persistent big tiles -------------------------------------
        # x8's slot is reused by yTall (tag ring, bufs=1): x is dead once
        # the v GEMMs finish, just before the first transpose lands.
        x8 = pool.tile([128, 2, 3, 2, t], FP8, name="x8", tag="big", bufs=1)
        v_aug = [
            pool.tile([128, H, HD + 1], BF16, name=f"vaug{i}") for i in range(nt)
        ]
        y_nat = [pool.tile([128, C], BF16, name=f"ynat{j}") for j in range(nt)]

        for i in range(nt):
            ones = v_aug[i][:, :, HD : HD + 1]
            nc.vector.memset(ones, 1.0)

        # ---- input DMAs (3 queues: SP, ACT, Pool) ---------------------
        # PE's first matmul needs x8-hi (3 chunks) + waP8 pair 0, so those
        # lead the SP queue; lo chunks follow on ACT/Pool.
        waPs = []
        for p in range(NHP):
            waPs.append(
                pool.tile([128, 2, 3, 512], FP8, name=f"waP{p}", tag="waP", bufs=2)
            )
        nc.sync.dma_start(out=x8[:, 0, 0], in_=xT8_d[0, 0])
        nc.sync.dma_start(
            out=waPs[0], in_=waP8_d[0].rearrange("hl c p o -> p hl c o")
        )
        nc.sync.dma_start(out=x8[:, 0, 1], in_=xT8_d[0, 1])
        nc.scalar.dma_start(out=x8[:, 0, 2], in_=xT8_d[0, 2])
        for c in range(3):
            nc.gpsimd.dma_start(out=x8[:, 1, c], in_=xT8_d[1, c])
        nc.sync.dma_start(
            out=bias_qk, in_=b_attn[0 : 2 * C].rearrange("(c p) -> p c", p=128)
        )
        for p in range(1, NHP):
            nc.sync.dma_start(
                out=waPs[p], in_=waP8_d[p].rearrange("hl c p o -> p hl c o")
            )
        for hl in range(2):
            for c in range(3):
                nc.gpsimd.dma_start(out=waV8[:, hl, c], in_=waV8_d[hl, c])
        for hl in range(2):
            for c in range(3):
                nc.gpsimd.dma_start(out=wpT8[:, hl, c], in_=wpT8_d[hl, c])
        # bias broadcasts are not needed until the first drains; keep them
        # behind the x8-lo chunks on the Pool queue.
        bav = b_attn[2 * C : 3 * C].rearrange("(o c) -> o c", o=1)
        nc.gpsimd.dma_start(
            out=bias_v,
            in_=bass.AP(tensor=bav.tensor, offset=bav.offset, ap=[[0, 128]] + bav.ap[1:]),
        )
        bpv = b_proj[:].rearrange("(o c) -> o c", o=1)
        nc.gpsimd.dma_start(
            out=bias_p,
            in_=bass.AP(tensor=bpv.tensor, offset=bpv.offset, ap=[[0, 128]] + bpv.ap[1:]),
        )

        # ---- PE warmup (p-state ramp while input DMAs land) -----------
        for _ in range(24):
            pw = psum.tile([128, 512], F32, name="ps_s", tag="s", bufs=2)
            nc.tensor.matmul(pw[:, 0:128], warm, warm, start=True, stop=True)

        qT = {}
        kT = {}
        TERMS = ((0, 0), (1, 0), (0, 1))  # (x_hl, w_hl): hi@hi + lo@hi + hi@lo

        def emit_qk(p):
            # inner order q0,q1,k0,k1: QK(p) i=0 needs q fully + k cols 0:512
            # only, so it can start one DVE-drain after the 3rd group.
            qT[p] = pool.tile([128, t], BF16, name=f"qT{p}", tag="qT", bufs=2)
            kT[p] = pool.tile([128, t], BF16, name=f"kT{p}", tag="kT", bufs=2)
            for qk in range(2):
                dst = qT[p] if qk == 0 else kT[p]
                ogidx = p if qk == 0 else NCC + p
                for (s, w) in _pieces(t):
                    pq = psum.tile([128, 512], F32, name="ps_mm", tag="mm", bufs=4)
                    for ti, (xh, wh) in enumerate(TERMS):
                        for c in range(3):
                            wsl = waPs[p][:, wh, c, :]
                            nc.tensor.matmul(
                                pq[:, :w],
                                bass.AP(
                                    tensor=wsl.tensor,
                                    offset=wsl.offset + qk * 128,
                                    ap=[wsl.ap[0], [256, 2], [1, 128]],
                                ),
                                x8[:, xh, c, :, s : s + w],
                                start=(ti == 0 and c == 0),
                                stop=(ti == 2 and c == 2),
                                perf_mode=DR,
                            )
                    nc.vector.tensor_scalar(
                        dst[:, s : s + w],
                        pq[:, :w],
                        1.0 / W8SCALE,
                        bias_qk[:, ogidx : ogidx + 1],
                        op0=mybir.AluOpType.mult,
                        op1=mybir.AluOpType.add,
                    )

        eS = {}

        def emit_qk_s(p, i_range):
            """S^T chunks + exp for pair p over the given tk-chunks.

            e tiles are [128, 2, w] (head-major). i<4 uses one 2-bank psum +
            one exp per head; i>=4 packs both heads into one psum tile
            (banks 0/1) and does a single gap-skipping exp for both.
            """
            for i in i_range:
                w = t - 128 * i
                e = pool.tile([128, 2, w], BF16, name=f"e{p}_{i}")
                eS[(p, i)] = e
                if w > 512:
                    for hh in range(2):
                        ps = psum.tile([128, 1024], F32, name="ps_s", tag="s", bufs=2)
                        for (s, wp) in _pieces(w):
                            tq0 = 128 * i + s
                            nc.tensor.matmul(
                                ps[:, s : s + wp],
                                kT[p][hh * 64 : hh * 64 + 64, 128 * i : 128 * (i + 1)],
                                qT[p][hh * 64 : hh * 64 + 64, tq0 : tq0 + wp],
                                start=True,
                                stop=True,
                            )
                        nc.scalar.activation(
                            e[:, hh, :],
                            ps[:, 0:w],
                            mybir.ActivationFunctionType.Exp,
                            bias=0.0,
                            scale=1.0 / float(np.sqrt(HD)),
                        )
                else:
                    ps = psum.tile([128, 1024], F32, name="ps_s", tag="s", bufs=2)
                    for hh in range(2):
                        nc.tensor.matmul(
                            ps[:, hh * 512 : hh * 512 + w],
                            kT[p][hh * 64 : hh * 64 + 64, 128 * i : 128 * (i + 1)],
                            qT[p][hh * 64 : hh * 64 + 64, 128 * i : t],
                            start=True,
                            stop=True,
                        )
                    nc.scalar.activation(
                        e,
                        bass.AP(
                            tensor=ps.tensor, offset=ps.offset, ap=[ps.ap[0], [512, 2], [1, w]]
                        ),
                        mybir.ActivationFunctionType.Exp,
                        bias=0.0,
                        scale=1.0 / float(np.sqrt(HD)),
                    )
                # causal mask on the diagonal 128x128 block (keep tk<=tq)
                for hh in range(2):
                    nc.gpsimd.tensor_mul(e[:, hh, 0:128], e[:, hh, 0:128], tri)

        def emit_v(it):
            for (s, w) in _pieces(C):
                pv = psum.tile([128, 512], F32, name="ps_mm", tag="mm", bufs=4)
                for ti, (xh, wh) in enumerate(TERMS):
                    for c in range(3):
                        nc.tensor.matmul(
                            pv[:, :w],
                            x8[:, xh, c, :, it * 128 : (it + 1) * 128],
                            waV8[:, wh, c, :, s : s + w],
                            start=(ti == 0 and c == 0),
                            stop=(ti == 2 and c == 2),
                            perf_mode=DR,
                        )
                nh = w // HD
                h0 = s // HD
                nc.vector.scalar_tensor_tensor(
                    out=v_aug[it][:, h0 : h0 + nh, 0:HD],
                    in0=pv[:, :w].rearrange("p (h d) -> p h d", d=HD),
                    scalar=1.0 / W8SCALE,
                    in1=bias_v[:, s : s + w].rearrange("p (h d) -> p h d", d=HD),
                    op0=mybir.AluOpType.mult,
                    op1=mybir.AluOpType.add,
                )

        # Schedule: exp starts ASAP (QK(0) i=0,1 right after qk(0)); qk/v
        # GEMMs are scattered between QK phases as PE filler so the PE tracks
        # ACT's exp pace (~10us/pair) without stalling on the ps_s ring.
        emit_qk(0)
        emit_qk_s(0, range(0, 2))
        emit_qk(1)
        emit_qk_s(0, range(2, nt))
        emit_qk(2)
        emit_qk_s(1, range(nt))
        emit_v(0)
        emit_v(1)
        emit_qk(3)
        emit_qk_s(2, range(nt))
        emit_v(2)
        emit_v(3)
        emit_qk(4)
        emit_qk_s(3, range(nt))
        emit_v(4)
        emit_v(5)
        emit_qk(5)
        emit_qk_s(4, range(nt))
        emit_v(6)
        emit_v(7)

        # ---- PV (j-major) + projection, software-pipelined ------------
        yTall = pool.tile([128, NCC, t], BF16, name="yTbf")
        yT8 = pool.tile([128, 2, 3, 2, t], FP8, name="yT8", tag="big", bufs=1)

        def emit_pv(j, half):
            py = psum.tile([128, 512], F32, name="ps_y", tag="mm", bufs=4)
            for hl in range(6):
                head = 6 * half + hl
                p, hh = head // 2, head % 2
                for i in range(j + 1):
                    nc.tensor.matmul(
                        py[:, hl * 65 : hl * 65 + 65],
                        eS[(p, i)][:, hh, 128 * (j - i) : 128 * (j - i) + 128],
                        v_aug[i][:, head, :],
                        start=(i == 0),
                        stop=(i == j),
                    )
            rcp = singles.tile([128, 6], F32, name="rcp", tag="rcp", bufs=4)
            nc.vector.reciprocal(
                rcp,
                bass.AP(tensor=py.tensor, offset=py.offset + HD, ap=[py.ap[0], [65, 6]]),
            )
            nc.vector.tensor_tensor(
                out=bass.AP(
                    tensor=y_nat[j].tensor,
                    offset=y_nat[j].offset + half * 384,
                    ap=[y_nat[j].ap[0], [64, 6], [1, 64]],
                ),
                in0=bass.AP(
                    tensor=py.tensor, offset=py.offset, ap=[py.ap[0], [65, 6], [1, 64]]
                ),
                in1=bass.AP(
                    tensor=rcp.tensor, offset=rcp.offset, ap=[rcp.ap[0], [1, 6], [0, 64]]
                ),
                op=mybir.AluOpType.mult,
            )

        def emit_proj(it):
            out_sb = singles.tile([128, C], F32, name="out_sb", tag="osb", bufs=2)
            for pi, (s, w) in enumerate(_pieces(C)):
                po = psum.tile([128, 512], F32, name="ps_o", tag="mm", bufs=4)
                for ti, (yh, wh) in enumerate(TERMS):
                    for c in range(3):
                        nc.tensor.matmul(
                            po[:, :w],
                            yT8[:, yh, c, :, it * 128 : (it + 1) * 128],
                            wpT8[:, wh, c, :, s : s + w],
                            start=(ti == 0 and c == 0),
                            stop=(ti == 2 and c == 2),
                            perf_mode=DR,
                        )
                if nobias:
                    # b_proj==0 fast path: drain on the (idle) ACT engine
                    nc.scalar.activation(
                        out_sb[:, s : s + w],
                        po[:, :w],
                        mybir.ActivationFunctionType.Copy,
                        bias=0.0,
                        scale=1.0 / W8SCALE,
                    )
                else:
                    nc.vector.scalar_tensor_tensor(
                        out=out_sb[:, s : s + w],
                        in0=po[:, :w],
                        scalar=1.0 / W8SCALE,
                        in1=bias_p[:, s : s + w],
                        op0=mybir.AluOpType.mult,
                        op1=mybir.AluOpType.add,
                    )
                # drain each piece to HBM as soon as it lands (SP/Pool split)
                eng = nc.sync if pi == 0 else nc.gpsimd
                eng.dma_start(
                    out=out[it * 128 : (it + 1) * 128, s : s + w],
                    in_=out_sb[:, s : s + w],
                )

        # QK(5)'s ps_s ring drains at ACT's exp pace, so interleave the PV
        # groups of heads 0-5 (pairs 0-2, whose exps are long done) as PE
        # filler; then pipeline pair-5-dependent PV + projection, with proj
        # staggered 3 behind PVh1 to cover the normalize+transpose latency.
        for jj in range(4):
            emit_qk_s(5, range(2 * jj, 2 * jj + 2))
            emit_pv(2 * jj, 0)
            emit_pv(2 * jj + 1, 0)
        def emit_split(j):
            # yT8 hi/lo from the bf16 transpose chunk (c-order matches the
            # DoubleRow row map r = 256c + 128k + p)
            src = yTall[:, :, j * 128 : (j + 1) * 128]
            hi = yT8[:, 0, :, :, j * 128 : (j + 1) * 128]
            nc.vector.tensor_scalar(
                hi, src, 1.0, None, op0=mybir.AluOpType.mult
            )
            nc.gpsimd.tensor_tensor(
                out=yT8[:, 1, :, :, j * 128 : (j + 1) * 128],
                in0=src,
                in1=hi,
                op=mybir.AluOpType.subtract,
            )

        for j in range(nt):
            emit_pv(j, 1)
            nc.sync.dma_start_transpose(
                yTall[:, :, j * 128 : (j + 1) * 128], y_nat[j]
            )
            emit_split(j)
            if j >= 4:
                emit_proj(j - 4)
        for it in range(nt - 4, nt):
            emit_proj(it)


_NC_CACHE = {}


def get_nc(t=T, nobias=True):
    key = (t, nobias)
    if key not in _NC_CACHE:
        _NC_CACHE[key] = build_attention_core(t, nobias=nobias)
    return _NC_CACHE[key]


def _to_bf16(a):
    import ml_dtypes

    return np.ascontiguousarray(np.asarray(a, dtype=np.float32)).astype(
        ml_dtypes.bfloat16
    )


def _split_fp8(a):
    """hi/lo error-compensated fp8e4 split: a ~= hi + lo."""
    import ml_dtypes

    f8 = ml_dtypes.float8_e4m3
    hi = a.astype(f8)
    lo = (a - hi.astype(np.float32)).astype(f8)
    return hi, lo


def _pack_dr(a, ncols):
    """[C, ncols] fp8 -> [3, 128, 2, ncols] DoubleRow layout
    (contraction row r = 256*chunk + 128*ktile + part)."""
    return np.ascontiguousarray(a.reshape(3, 2, 128, ncols).transpose(0, 2, 1, 3))


def host_prep(inputs):
    """Transpose + cast + pack weights/x on the host for the device program."""
    x = np.asarray(inputs["x"], dtype=np.float32)
    b_attn = np.ascontiguousarray(inputs["b_attn"], dtype=np.float32)
    b_proj = np.ascontiguousarray(inputs["b_proj"], dtype=np.float32)
    waT = np.asarray(inputs["w_attn"], dtype=np.float32).T * W8SCALE  # [C, 3C]
    waP8 = np.stack(
        [
            np.stack(
                [
                    _pack_dr(h, 256).reshape(3, 128, 512)
                    for h in _split_fp8(
                        np.concatenate(
                            [
                                waT[:, p * 128 : (p + 1) * 128],
                                waT[:, (NCC + p) * 128 : (NCC + p + 1) * 128],
                            ],
                            axis=1,
                        )
                    )
                ]
            )
            for p in range(NHP)
        ]
    )  # [6, 2, 3, 128, 2, 256]
    waV8 = np.stack([_pack_dr(h, C) for h in _split_fp8(waT[:, 2 * C : 3 * C])])
    wpT8 = np.stack(
        [
            _pack_dr(h, C)
            for h in _split_fp8(np.asarray(inputs["w_proj"], dtype=np.float32).T * W8SCALE)
        ]
    )  # [2, 3, 128, 2, C]
    return [
        {
            "xT8": np.stack(
                [_pack_dr(h, x.shape[1]) for h in _split_fp8(x[b].T)]
            ),  # [2, 3, 128, 2, T]
            "waP8": waP8,
            "waV8": waV8,
            "b_attn": b_attn,
            "wpT8": wpT8,
            "b_proj": b_proj,
        }
        for b in range(x.shape[0])
    ]


def kernel(**inputs):
    from concourse.bass_utils import run_bass_kernel_spmd

    x = inputs["x"]
    B, t, _ = x.shape
    assert B == N_CORES
    in_maps = host_prep(inputs)
    nobias = not (
        np.any(np.asarray(inputs["b_attn"])) or np.any(np.asarray(inputs["b_proj"]))
    )
    nc = get_nc(t, nobias)
    res = run_bass_kernel_spmd(nc, in_maps, core_ids=list(range(N_CORES)))
    return np.stack([res.results[b]["out"] for b in range(B)]).astype(np.float32)
